# revision 15
# baseline (speedup 1.0000x reference)
"""BiLSTM-CRF Viterbi decode kernel for 8 Trainium2 NeuronCores.

Problem shapes (hardcoded): V=50257, E=128, H=128, T=12, B=64, S=512.

Sharding: data-parallel over batch, 8 sequences per core. Each core runs
the forward and backward LSTM scans with both directions' per-step
elementwise work merged into shared instructions (backward inputs are
stored time-reversed so one slice covers both directions), computes
emissions, and runs the CRF Viterbi forward scan, emitting the per-step
score series. The host does constant prep (bias folding, gate
reordering, tanh-as-sigmoid scaling) and the integer backtrace.
"""

import os

import numpy as np

V, E, H, T, B, S = 50257, 128, 128, 12, 64, 512
NCORES = 8
PB = B // NCORES          # batch per core = 8
NBLK = (S * PB) // 128    # 128-token gather/matmul blocks = 32
G4 = 4 * H                # 512 gate rows per direction
# gate order used on device within one direction: i, f, o, g
# (PyTorch order is i, f, g, o)
GATE_PERM = [0, 1, 3, 2]
# chunk positions in the merged 8-chunk gate tile
POS = {"f": [0, 1, 2, 6], "b": [3, 4, 5, 7]}

_PROGRAM_CACHE = {}
LAST_RESULT = None
DT_MM = os.environ.get("KDT", "f16")


def _np_dt(dt_mm):
    import ml_dtypes
    return {"f32": np.float32, "f16": np.float16,
            "bf16": ml_dtypes.bfloat16}[dt_mm]


def build_program(s_len=S, pb=PB, dt_mm="f16", dt_xg="f16"):
    """Build the Bass/Tile SPMD program for one core."""
    import concourse.bacc as bacc
    import concourse.bass as bass
    import concourse.mybir as mybir
    import concourse.tile as tile

    fp32 = mybir.dt.float32
    DTMM = {"f32": mybir.dt.float32, "f16": mybir.dt.float16,
            "bf16": mybir.dt.bfloat16}[dt_mm]
    DTXG = {"f32": mybir.dt.float32, "f16": mybir.dt.float16,
            "bf16": mybir.dt.bfloat16}[dt_xg]
    AF = mybir.ActivationFunctionType
    ALU = mybir.AluOpType
    AX = mybir.AxisListType

    nblk = (s_len * pb) // 128
    ntok = s_len * pb

    nc = bacc.Bacc("TRN2", target_bir_lowering=False, debug=False)

    # ---- DRAM I/O ----
    d_emb = nc.dram_tensor("emb_w", [V, E], DTMM, kind="ExternalInput")
    d_ids = nc.dram_tensor("ids", [128, nblk], mybir.dt.int32,
                           kind="ExternalInput")
    d_idsr = nc.dram_tensor("ids_rev", [128, nblk], mybir.dt.int32,
                            kind="ExternalInput")
    d_wih = {}
    d_whh = {}
    d_bm = {}
    for d in ("f", "b"):
        d_wih[d] = nc.dram_tensor(f"wih_{d}", [E, G4], DTMM,
                                  kind="ExternalInput")
        d_whh[d] = nc.dram_tensor(f"whh_{d}", [H, G4], DTMM,
                                  kind="ExternalInput")
        d_bm[d] = nc.dram_tensor(f"biasmat_{d}", [4, 128], DTMM,
                                 kind="ExternalInput")
    d_ind = nc.dram_tensor("bias_ind", [4, 4 * 128], DTMM,
                           kind="ExternalInput")
    d_wof = nc.dram_tensor("wout_f", [H, T], DTMM, kind="ExternalInput")
    d_wob = nc.dram_tensor("wout_b", [H, T], DTMM, kind="ExternalInput")
    d_ident = nc.dram_tensor("ident", [128, 128], DTMM, kind="ExternalInput")
    d_start = nc.dram_tensor("start_t", [pb, T], fp32, kind="ExternalInput")
    d_trep = nc.dram_tensor("transrep", [pb, T * T], fp32,
                            kind="ExternalInput")
    d_scores = nc.dram_tensor("scores", [pb, s_len, T], fp32,
                              kind="ExternalOutput")

    with tile.TileContext(nc) as tc:
        with (
            tc.tile_pool(name="singles", bufs=1) as singles,
            tc.tile_pool(name="big", bufs=1) as big,
            tc.tile_pool(name="crf", bufs=2) as crf,
        ):
            # ---- load constants ----
            sb_wih = {}
            sb_whh = {}
            sb_bm = {}
            for d in ("f", "b"):
                sb_wih[d] = singles.tile([E, G4], DTMM, name=f"wih{d}")
                nc.sync.dma_start(out=sb_wih[d][:], in_=d_wih[d].ap())
                sb_whh[d] = singles.tile([H, G4], DTMM, name=f"whh{d}")
                nc.sync.dma_start(out=sb_whh[d][:], in_=d_whh[d].ap())
                sb_bm[d] = singles.tile([4, 128], DTMM, name=f"bm{d}")
                nc.sync.dma_start(out=sb_bm[d][:], in_=d_bm[d].ap())
            sb_ind = singles.tile([4, 4 * 128], DTMM, name="ind")
            nc.sync.dma_start(out=sb_ind[:], in_=d_ind.ap())
            sb_wof = singles.tile([H, T], DTMM, name="wof")
            nc.sync.dma_start(out=sb_wof[:], in_=d_wof.ap())
            sb_wob = singles.tile([H, T], DTMM, name="wob")
            nc.sync.dma_start(out=sb_wob[:], in_=d_wob.ap())
            sb_ident = singles.tile([128, 128], DTMM, name="ident")
            nc.sync.dma_start(out=sb_ident[:], in_=d_ident.ap())
            sb_start = singles.tile([pb, T], fp32, name="start")
            nc.sync.dma_start(out=sb_start[:], in_=d_start.ap())
            sb_trep = singles.tile([pb, T * T], fp32, name="trep")
            nc.sync.dma_start(out=sb_trep[:], in_=d_trep.ap())
            sb_ids = singles.tile([128, nblk], mybir.dt.int32, name="ids")
            nc.sync.dma_start(out=sb_ids[:], in_=d_ids.ap())
            sb_idsr = singles.tile([128, nblk], mybir.dt.int32, name="idsr")
            nc.sync.dma_start(out=sb_idsr[:], in_=d_idsr.ap())

            # ---- persistent big buffers ----
            # xg: merged gate pre-activations, 8 chunks
            # (i_f,f_f,o_f,i_b,f_b,o_b,g_f,g_b); backward stored
            # time-reversed so step t reads one contiguous slice.
            xg = big.tile([128, 8, ntok], DTXG, name="xg")
            hT = {d: big.tile([128, s_len, pb], DTMM, name=f"hT{d}")
                  for d in ("f", "b")}
            em_sb = big.tile([pb, s_len, T], fp32, name="emsb")
            score = big.tile([pb, s_len, T], fp32, name="score")

            # ---- phase 1+2: gather embeddings, transpose to [E, tok] ----
            with (
                tc.tile_pool(name="gather", bufs=4) as gather,
                tc.tile_pool(name="tps", bufs=2, space="PSUM") as tps,
                tc.tile_pool(name="xt", bufs=1) as xtp,
                tc.tile_pool(name="xgps", bufs=2, space="PSUM") as xgps,
            ):
                xT = {"f": xtp.tile([128, nblk, 128], DTMM, name="xTf"),
                      "b": xtp.tile([128, nblk, 128], DTMM, name="xTb")}
                idt = {"f": sb_ids, "b": sb_idsr}
                for k in range(nblk):
                    for d in ("f", "b"):
                        ge = gather.tile([128, E], DTMM, tag=f"ge{d}",
                                         name=f"ge{d}")
                        nc.gpsimd.indirect_dma_start(
                            out=ge[:],
                            out_offset=None,
                            in_=d_emb.ap(),
                            in_offset=bass.IndirectOffsetOnAxis(
                                ap=idt[d][:, k:k + 1], axis=0),
                        )
                        pt = tps.tile([128, 128], DTMM, tag=f"pt{d}",
                                      name=f"pt{d}")
                        nc.tensor.transpose(out=pt[:], in_=ge[:],
                                            identity=sb_ident[:])
                        if (k + (d == "b")) % 2 == 0:
                            nc.vector.tensor_copy(xT[d][:, k, :], pt[:])
                        else:
                            nc.scalar.copy(xT[d][:, k, :], pt[:])

                # ---- phase 3: input gates xg = Wih @ x + bias ----
                for k in range(nblk):
                    for d in ("f", "b"):
                        ps = xgps.tile([128, 4, 128], fp32, tag="xgps",
                                       name="xgps")
                        nc.tensor.matmul(
                            ps[:].rearrange("p a b -> p (a b)"),
                            sb_bm[d][:],
                            sb_ind[:],
                            start=True, stop=False,
                            skip_group_check=True,
                        )
                        for j in range(4):
                            nc.tensor.matmul(
                                ps[:, j, :],
                                sb_wih[d][:, j * 128:(j + 1) * 128],
                                xT[d][:, k, :],
                                start=False, stop=(j == 3),
                                skip_group_check=True,
                            )
                        p0 = POS[d][0]
                        kb = slice(k * 128, (k + 1) * 128)
                        if (k + (d == "b")) % 2 == 0:
                            nc.scalar.copy(xg[:, p0:p0 + 3, kb], ps[:, 0:3, :])
                            nc.vector.tensor_copy(xg[:, POS[d][3], kb],
                                                  ps[:, 3, :])
                        else:
                            nc.vector.tensor_copy(xg[:, p0:p0 + 3, kb],
                                                  ps[:, 0:3, :])
                            nc.scalar.copy(xg[:, POS[d][3], kb], ps[:, 3, :])

            # ---- phase 4: merged fwd+bwd LSTM scan ----
            with (
                tc.tile_pool(name="gps", bufs=2, space="PSUM") as gpsp,
                tc.tile_pool(name="state", bufs=1) as state,
                tc.tile_pool(name="step", bufs=3) as step,
            ):
                c_tiles = [state.tile([128, 2 * pb], fp32, name=f"c{i}")
                           for i in range(2)]

                for t in range(s_len):
                    tkb = s_len - 1 - t   # backward time index
                    c_prev = c_tiles[(t + 1) % 2]
                    c_new = c_tiles[t % 2]

                    gs = step.tile([128, 8 * pb], fp32, tag="gs", name="gs")
                    gs3 = gs[:].rearrange("p (a b) -> p a b", a=8)
                    xsl = xg[:, :, t * pb:(t + 1) * pb]
                    if t > 0:
                        ps = gpsp.tile([128, 8, pb], fp32, tag="g", name="g")
                        for d in ("f", "b"):
                            h_prev = (hT["f"][:, t - 1, :] if d == "f"
                                      else hT["b"][:, tkb + 1, :])
                            for j in range(4):
                                nc.tensor.matmul(
                                    ps[:, POS[d][j], :],
                                    sb_whh[d][:, j * 128:(j + 1) * 128],
                                    h_prev,
                                    start=True, stop=True,
                                    skip_group_check=True,
                                )
                        nc.vector.tensor_add(gs3, ps[:], xsl)
                    else:
                        nc.vector.tensor_copy(gs3, xsl)

                    # single sigmoid; g-gates are pre-scaled x2 so
                    # tanh(x) = 2*sigmoid(2x) - 1
                    a_all = step.tile([128, 8 * pb], fp32, tag="aa",
                                      name="aa")
                    nc.scalar.activation(a_all[:], gs[:], AF.Sigmoid)
                    ba = a_all[:]
                    strided = lambda off: bass.AP(
                        ba.tensor, ba.offset + off,
                        [[8 * pb, 128], [3 * pb, 2], [1, pb]])
                    a_i = strided(0)
                    a_f = strided(pb)
                    a_g = ba[:, 6 * pb:8 * pb].rearrange(
                        "p (u b) -> p u b", u=2)

                    c_new3 = c_new[:].rearrange("p (u b) -> p u b", u=2)
                    m2p = step.tile([128, 2 * pb], fp32, tag="m2p",
                                    name="m2p")
                    m2p3 = m2p[:].rearrange("p (u b) -> p u b", u=2)
                    # m2' = (sig(2g) - 0.5) * sig(i)  == i*(tanh(g))/2
                    nc.vector.scalar_tensor_tensor(
                        out=m2p3, in0=a_g, scalar=0.5, in1=a_i,
                        op0=ALU.subtract, op1=ALU.mult)
                    if t > 0:
                        m1 = step.tile([128, 2 * pb], fp32, tag="m1",
                                       name="m1")
                        m13 = m1[:].rearrange("p (u b) -> p u b", u=2)
                        nc.gpsimd.tensor_mul(m13, a_f, c_prev3_prev)
                        nc.vector.scalar_tensor_tensor(
                            out=c_new3, in0=m2p3, scalar=2.0, in1=m13,
                            op0=ALU.mult, op1=ALU.add)
                    else:
                        nc.vector.tensor_scalar_mul(c_new[:], m2p[:], 2.0)
                    c_prev3_prev = c_new3

                    t_c = step.tile([128, 2 * pb], fp32, tag="tc", name="tc")
                    nc.scalar.activation(t_c[:], c_new[:], AF.Tanh)
                    nc.vector.tensor_mul(hT["f"][:, t, :],
                                         ba[:, 2 * pb:3 * pb],
                                         t_c[:, 0:pb])
                    nc.gpsimd.tensor_mul(hT["b"][:, tkb, :],
                                         ba[:, 5 * pb:6 * pb],
                                         t_c[:, pb:2 * pb])

            # ---- phase 5: emissions into PSUM [(s16,b), blk, T] ----
            from contextlib import ExitStack
            _emctx = ExitStack()
            empool = _emctx.enter_context(
                tc.tile_pool(name="empool", bufs=1, space="PSUM"))
            em_ps = empool.tile([128, nblk, T], fp32, name="emps")
            for k in range(nblk):
                nc.tensor.matmul(
                    em_ps[:, k, :],
                    hT["f"][:].rearrange("p s b -> p (s b)")
                    [:, k * 128:(k + 1) * 128],
                    sb_wof[:],
                    start=True, stop=False, skip_group_check=True,
                )
                nc.tensor.matmul(
                    em_ps[:, k, :],
                    hT["b"][:].rearrange("p s b -> p (s b)")
                    [:, k * 128:(k + 1) * 128],
                    sb_wob[:],
                    start=False, stop=True, skip_group_check=True,
                )

            # stage PSUM -> SBUF, then reshuffle [(s16,b) part, blk, T]
            # -> em_sb [b part, s, T]
            em_stage = big.tile([128, nblk, T], fp32, name="emstage")
            half = (nblk // 2) * T
            nc.vector.tensor_copy(
                em_stage[:].rearrange("p a b -> p (a b)")[:, 0:half],
                em_ps[:].rearrange("p a b -> p (a b)")[:, 0:half])
            nc.scalar.copy(
                em_stage[:].rearrange("p a b -> p (a b)")[:, half:nblk * T],
                em_ps[:].rearrange("p a b -> p (a b)")[:, half:nblk * T])
            s16cnt = 128 // pb
            _emctx.close()
            pitch = nblk * T
            for s16 in range(s16cnt):
                src_ap = bass.AP(
                    em_stage[:].tensor,
                    em_stage[:].offset + s16 * pb * pitch,
                    [[pitch, pb], [T, nblk], [1, T]],
                )
                dst_ap = bass.AP(
                    em_sb[:].tensor,
                    em_sb[:].offset + s16 * T,
                    [[s_len * T, pb], [s16cnt * T, nblk], [1, T]],
                )
                nc.sync.dma_start(out=dst_ap, in_=src_ap)

            # ---- phase 6: CRF Viterbi forward ----
            nc.vector.tensor_add(score[:, 0, :], em_sb[:, 0, :],
                                 sb_start[:])
            for t in range(1, s_len):
                tmp = crf.tile([pb, T * T], fp32, tag="tmp", name="tmp")
                prev = score[:, t - 1, :].rearrange(
                    "p (o c) -> p o c", o=1).to_broadcast([pb, T, T])
                nc.vector.tensor_tensor(
                    out=tmp[:], in0=prev,
                    in1=sb_trep[:], op=ALU.add)
                mx = crf.tile([pb, T], fp32, tag="mx", name="mx")
                nc.vector.tensor_reduce(
                    out=mx[:],
                    in_=tmp[:].rearrange("p (c q) -> p c q", q=T),
                    axis=AX.X, op=ALU.max)
                nc.vector.tensor_add(score[:, t, :], mx[:],
                                     em_sb[:, t, :])

            nc.sync.dma_start(out=d_scores.ap(), in_=score[:])

    nc.compile()
    return nc


def _prep_host(inputs, dt_np, s_len=S, nblk=NBLK):
    """Build per-core in_maps from full inputs."""
    x = np.asarray(inputs["x"])
    emb = np.asarray(inputs["emb"], dtype=np.float32)
    w_out = np.asarray(inputs["w_out"], dtype=np.float32)
    b_out = np.asarray(inputs["b_out"], dtype=np.float32)
    start = np.asarray(inputs["start"], dtype=np.float32)
    trans = np.asarray(inputs["trans"], dtype=np.float32)

    def perm_rows(w):
        chunks = [w[i * H:(i + 1) * H] for i in range(4)]
        return np.concatenate([chunks[i] for i in GATE_PERM], axis=0)

    shared = {"emb_w": emb.astype(dt_np)}
    for d, (wi, wh, bb) in (("f", ("w_ih_f", "w_hh_f", "b_f")),
                            ("b", ("w_ih_b", "w_hh_b", "b_b"))):
        wih = perm_rows(np.asarray(inputs[wi], dtype=np.float32)).copy()
        whh = perm_rows(np.asarray(inputs[wh], dtype=np.float32)).copy()
        bias = perm_rows(np.asarray(inputs[bb],
                                    dtype=np.float32).reshape(-1, 1))[:, 0]
        bias = bias.copy()
        # tanh-as-sigmoid: scale g-gate (chunk 3) pre-activations by 2
        wih[3 * H:] *= 2.0
        whh[3 * H:] *= 2.0
        bias[3 * H:] *= 2.0
        shared[f"wih_{d}"] = np.ascontiguousarray(wih.T).astype(dt_np)
        shared[f"whh_{d}"] = np.ascontiguousarray(whh.T).astype(dt_np)
        shared[f"biasmat_{d}"] = bias.reshape(4, 128).astype(dt_np)
    ind = np.zeros((4, 4, 128), dtype=np.float32)
    for j in range(4):
        ind[j, j, :] = 1.0
    shared["bias_ind"] = ind.reshape(4, 512).astype(dt_np)
    shared["wout_f"] = np.ascontiguousarray(w_out[:, :H].T).astype(dt_np)
    shared["wout_b"] = np.ascontiguousarray(w_out[:, H:].T).astype(dt_np)
    shared["ident"] = np.eye(128, dtype=np.float32).astype(dt_np)
    shared["start_t"] = np.tile((start + b_out)[None, :], (PB, 1)).astype(
        np.float32)
    trep = (trans + b_out[None, :]).T.reshape(-1)  # [(c,p)]
    shared["transrep"] = np.tile(trep[None, :], (PB, 1)).astype(np.float32)

    in_maps = []
    for k in range(NCORES):
        xc = x[k * PB:(k + 1) * PB]              # [pb, s]
        ids = np.ascontiguousarray(
            xc.T.reshape(-1).reshape(nblk, 128).T).astype(np.int32)
        idsr = np.ascontiguousarray(
            xc[:, ::-1].T.reshape(-1).reshape(nblk, 128).T).astype(np.int32)
        m = dict(shared)
        m["ids"] = ids
        m["ids_rev"] = idsr
        in_maps.append(m)
    return in_maps


def _host_finalize(scores, trans, end):
    """scores [B, S, T] f32 -> (path int32 [B,S], best f32 [B])."""
    final = scores[:, -1, :] + end[None, :]
    last = np.argmax(final, axis=-1).astype(np.int32)
    best = final.max(axis=-1).astype(np.float32)
    path = np.empty((scores.shape[0], scores.shape[1]), dtype=np.int32)
    path[:, -1] = last
    tag = last
    for t in range(scores.shape[1] - 1, 0, -1):
        val = scores[:, t - 1, :] + trans[:, tag].T   # [B, T(prev)]
        tag = np.argmax(val, axis=-1).astype(np.int32)
        path[:, t - 1] = tag
    return path, best


def _reference_np(inputs):
    """Exact numpy fallback (general mask)."""
    x = np.asarray(inputs["x"])
    mask = np.asarray(inputs["mask"])
    emb = np.asarray(inputs["emb"], np.float32)
    xt = emb[x].transpose(1, 0, 2)

    def lstm(xg, whh):
        h = np.zeros((xg.shape[1], whh.shape[1]), np.float32)
        c = np.zeros_like(h)
        hs = []
        sig = lambda z: 1.0 / (1.0 + np.exp(-z))
        for g_t in xg:
            g = g_t + h @ whh.T
            i, f, gg, o = np.split(g, 4, -1)
            c = sig(f) * c + sig(i) * np.tanh(gg)
            h = sig(o) * np.tanh(c)
            hs.append(h)
        return np.stack(hs)

    xg_f = xt @ np.asarray(inputs["w_ih_f"], np.float32).T + np.asarray(
        inputs["b_f"], np.float32)
    xg_b = xt[::-1] @ np.asarray(inputs["w_ih_b"], np.float32).T + np.asarray(
        inputs["b_b"], np.float32)
    h = np.concatenate([lstm(xg_f, np.asarray(inputs["w_hh_f"], np.float32)),
                        lstm(xg_b, np.asarray(inputs["w_hh_b"],
                                              np.float32))[::-1]], -1)
    em = h @ np.asarray(inputs["w_out"], np.float32).T + np.asarray(
        inputs["b_out"], np.float32)
    trans = np.asarray(inputs["trans"], np.float32)
    m = mask.T
    sc = np.asarray(inputs["start"], np.float32) + em[0]
    hist = []
    for t in range(1, em.shape[0]):
        tot = sc[:, :, None] + trans[None]
        best, idx = tot.max(1), tot.argmax(1).astype(np.int32)
        hist.append(idx)
        sc = np.where(m[t][:, None], best + em[t], sc)
    final = sc + np.asarray(inputs["end"], np.float32)
    last = np.argmax(final, -1).astype(np.int32)
    best = final.max(-1)
    tags = [last]
    tag = last
    bidx = np.arange(x.shape[0])
    for t in range(em.shape[0] - 2, -1, -1):
        prev = hist[t][bidx, tag]
        tag = np.where(m[t + 1], prev, tag)
        tags.append(tag)
    path = np.stack(tags[::-1], 1).astype(np.int32)
    return path, best.astype(np.float32)


def kernel(**inputs):
    mask = np.asarray(inputs["mask"])
    if not mask.all():
        return _reference_np(inputs)

    dt_mm = DT_MM
    key = (S, PB, dt_mm, "f16")
    if key not in _PROGRAM_CACHE:
        _PROGRAM_CACHE[key] = build_program(S, PB, dt_mm, "f16")
    nc = _PROGRAM_CACHE[key]

    from concourse.bass_utils import run_bass_kernel_spmd
    in_maps = _prep_host(inputs, _np_dt(dt_mm))
    res = run_bass_kernel_spmd(nc, in_maps, core_ids=list(range(NCORES)))
    global LAST_RESULT
    LAST_RESULT = res
    scores = np.concatenate([res.results[k]["scores"]
                             for k in range(NCORES)], axis=0)
    trans = np.asarray(inputs["trans"], np.float32)
    end = np.asarray(inputs["end"], np.float32)
    path, best = _host_finalize(scores, trans, end)
    return path, best


# revision 16
# speedup vs baseline: 1.1764x; 1.1764x over previous
"""BiLSTM-CRF Viterbi decode kernel for 8 Trainium2 NeuronCores.

Problem shapes (hardcoded): V=50257, E=128, H=128, T=12, B=64, S=512.

Sharding: data-parallel over batch, 8 sequences per core. Each core runs
the forward and backward LSTM scans with both directions' per-step
elementwise work merged into shared instructions (backward inputs are
stored time-reversed so one slice covers both directions), computes
emissions, and runs the CRF Viterbi forward scan, emitting the per-step
score series. The host does constant prep (bias folding, gate
reordering, tanh-as-sigmoid scaling) and the integer backtrace.
"""

import os

import numpy as np

V, E, H, T, B, S = 50257, 128, 128, 12, 64, 512
NCORES = 8
PB = B // NCORES          # batch per core = 8
NBLK = (S * PB) // 128    # 128-token gather/matmul blocks = 32
G4 = 4 * H                # 512 gate rows per direction
# gate order used on device within one direction: i, f, o, g
# (PyTorch order is i, f, g, o)
GATE_PERM = [0, 1, 3, 2]
# chunk positions in the merged 8-chunk gate tile
POS = {"f": [0, 1, 2, 6], "b": [3, 4, 5, 7]}

_PROGRAM_CACHE = {}
LAST_RESULT = None
DT_MM = os.environ.get("KDT", "f16")


def _np_dt(dt_mm):
    import ml_dtypes
    return {"f32": np.float32, "f16": np.float16,
            "bf16": ml_dtypes.bfloat16}[dt_mm]


def build_program(s_len=S, pb=PB, dt_mm="f16", dt_xg="f16"):
    """Build the Bass/Tile SPMD program for one core."""
    import concourse.bacc as bacc
    import concourse.bass as bass
    import concourse.mybir as mybir
    import concourse.tile as tile

    fp32 = mybir.dt.float32
    DTMM = {"f32": mybir.dt.float32, "f16": mybir.dt.float16,
            "bf16": mybir.dt.bfloat16}[dt_mm]
    DTXG = {"f32": mybir.dt.float32, "f16": mybir.dt.float16,
            "bf16": mybir.dt.bfloat16}[dt_xg]
    AF = mybir.ActivationFunctionType
    ALU = mybir.AluOpType
    AX = mybir.AxisListType

    nblk = (s_len * pb) // 128
    ntok = s_len * pb

    nc = bacc.Bacc("TRN2", target_bir_lowering=False, debug=False)

    # ---- DRAM I/O ----
    d_emb = nc.dram_tensor("emb_w", [V, E], DTMM, kind="ExternalInput")
    d_ids = nc.dram_tensor("ids", [128, nblk], mybir.dt.int32,
                           kind="ExternalInput")
    d_idsr = nc.dram_tensor("ids_rev", [128, nblk], mybir.dt.int32,
                            kind="ExternalInput")
    d_wih = {}
    d_whh = {}
    d_bm = {}
    for d in ("f", "b"):
        d_wih[d] = nc.dram_tensor(f"wih_{d}", [E, G4], DTMM,
                                  kind="ExternalInput")
        d_whh[d] = nc.dram_tensor(f"whh_{d}", [H, G4], DTMM,
                                  kind="ExternalInput")
        d_bm[d] = nc.dram_tensor(f"biasmat_{d}", [4, 128], DTMM,
                                 kind="ExternalInput")
    d_ind = nc.dram_tensor("bias_ind", [4, 4 * pb], DTMM,
                           kind="ExternalInput")
    d_wof = nc.dram_tensor("wout_f", [H, T], DTMM, kind="ExternalInput")
    d_wob = nc.dram_tensor("wout_b", [H, T], DTMM, kind="ExternalInput")
    d_ident = nc.dram_tensor("ident", [128, 128], DTMM, kind="ExternalInput")
    d_start = nc.dram_tensor("start_t", [pb, T], fp32, kind="ExternalInput")
    d_trep = nc.dram_tensor("transrep", [pb, T * T], fp32,
                            kind="ExternalInput")
    d_scores = nc.dram_tensor("scores", [pb, s_len, T], fp32,
                              kind="ExternalOutput")

    with tile.TileContext(nc) as tc:
        with (
            tc.tile_pool(name="singles", bufs=1) as singles,
            tc.tile_pool(name="big", bufs=1) as big,
            tc.tile_pool(name="crf", bufs=2) as crf,
        ):
            # ---- load constants ----
            sb_wih = {}
            sb_whh = {}
            sb_bm = {}
            for d in ("f", "b"):
                sb_wih[d] = singles.tile([E, G4], DTMM, name=f"wih{d}")
                nc.sync.dma_start(out=sb_wih[d][:], in_=d_wih[d].ap())
                sb_whh[d] = singles.tile([H, G4], DTMM, name=f"whh{d}")
                nc.sync.dma_start(out=sb_whh[d][:], in_=d_whh[d].ap())
                sb_bm[d] = singles.tile([4, 128], DTMM, name=f"bm{d}")
                nc.sync.dma_start(out=sb_bm[d][:], in_=d_bm[d].ap())
            sb_ind = singles.tile([4, 4 * pb], DTMM, name="ind")
            nc.sync.dma_start(out=sb_ind[:], in_=d_ind.ap())
            sb_wof = singles.tile([H, T], DTMM, name="wof")
            nc.sync.dma_start(out=sb_wof[:], in_=d_wof.ap())
            sb_wob = singles.tile([H, T], DTMM, name="wob")
            nc.sync.dma_start(out=sb_wob[:], in_=d_wob.ap())
            sb_ident = singles.tile([128, 128], DTMM, name="ident")
            nc.sync.dma_start(out=sb_ident[:], in_=d_ident.ap())
            sb_start = singles.tile([pb, T], fp32, name="start")
            nc.sync.dma_start(out=sb_start[:], in_=d_start.ap())
            sb_trep = singles.tile([pb, T * T], fp32, name="trep")
            nc.sync.dma_start(out=sb_trep[:], in_=d_trep.ap())
            sb_ids = singles.tile([128, nblk], mybir.dt.int32, name="ids")
            nc.sync.dma_start(out=sb_ids[:], in_=d_ids.ap())
            sb_idsr = singles.tile([128, nblk], mybir.dt.int32, name="idsr")
            nc.sync.dma_start(out=sb_idsr[:], in_=d_idsr.ap())

            # ---- persistent big buffers ----
            hT = {d: big.tile([128, s_len, pb], DTMM, name=f"hT{d}")
                  for d in ("f", "b")}
            em_sb = big.tile([pb, s_len, T], fp32, name="emsb")
            score = big.tile([pb, s_len, T], fp32, name="score")

            # ---- phase 1+2: gather embeddings, transpose to [E, tok] ----
            with (
                tc.tile_pool(name="gather", bufs=4) as gather,
                tc.tile_pool(name="tps", bufs=2, space="PSUM") as tps,
            ):
                xT = {"f": big.tile([128, nblk, 128], DTMM, name="xTf"),
                      "b": big.tile([128, nblk, 128], DTMM, name="xTb")}
                idt = {"f": sb_ids, "b": sb_idsr}
                for k in range(nblk):
                    for d in ("f", "b"):
                        ge = gather.tile([128, E], DTMM, tag=f"ge{d}",
                                         name=f"ge{d}")
                        nc.gpsimd.indirect_dma_start(
                            out=ge[:],
                            out_offset=None,
                            in_=d_emb.ap(),
                            in_offset=bass.IndirectOffsetOnAxis(
                                ap=idt[d][:, k:k + 1], axis=0),
                        )
                        pt = tps.tile([128, 128], DTMM, tag=f"pt{d}",
                                      name=f"pt{d}")
                        nc.tensor.transpose(out=pt[:], in_=ge[:],
                                            identity=sb_ident[:])
                        if (k + (d == "b")) % 2 == 0:
                            nc.vector.tensor_copy(xT[d][:, k, :], pt[:])
                        else:
                            nc.scalar.copy(xT[d][:, k, :], pt[:])

            # ---- phase 4: fwd+bwd LSTM scans (independent chains) ----
            # Per step and direction, PSUM accumulates
            # bias + W_ih@x_t + W_hh@h_{t-1} for all four gate chunks
            # (order i,f,o,g; g pre-scaled x2 for tanh-as-sigmoid).
            s16cnt = 128 // pb
            with (
                tc.tile_pool(name="gps_f", bufs=2, space="PSUM") as gps_f,
                tc.tile_pool(name="gps_b", bufs=2, space="PSUM") as gps_b,
                tc.tile_pool(name="state", bufs=1) as state,
                tc.tile_pool(name="step", bufs=4) as step,
            ):
                c_tiles = {d: [state.tile([128, pb], fp32, name=f"c{d}{i}")
                               for i in range(2)] for d in ("f", "b")}

                for t in range(s_len):
                    for d in ("f", "b"):
                        tk = t if d == "f" else s_len - 1 - t
                        gp = gps_f if d == "f" else gps_b
                        c_prev = c_tiles[d][(t + 1) % 2]
                        c_new = c_tiles[d][t % 2]

                        ps = gp.tile([128, 4, pb], fp32, tag=f"g{d}",
                                     name=f"g{d}")
                        nc.tensor.matmul(
                            ps[:].rearrange("p a b -> p (a b)"),
                            sb_bm[d][:],
                            sb_ind[:],
                            start=True, stop=False,
                            skip_group_check=True,
                        )
                        xrhs = xT[d][:, t // s16cnt,
                                     (t % s16cnt) * pb:
                                     (t % s16cnt + 1) * pb]
                        for j in range(4):
                            nc.tensor.matmul(
                                ps[:, j, :],
                                sb_wih[d][:, j * 128:(j + 1) * 128],
                                xrhs,
                                start=False, stop=(t == 0 and j == 3),
                                skip_group_check=True,
                            )
                        if t > 0:
                            h_prev = (hT["f"][:, t - 1, :] if d == "f"
                                      else hT["b"][:, tk + 1, :])
                            for j in range(4):
                                nc.tensor.matmul(
                                    ps[:, j, :],
                                    sb_whh[d][:, j * 128:(j + 1) * 128],
                                    h_prev,
                                    start=False, stop=(j == 3),
                                    skip_group_check=True,
                                )

                        # A = sigmoid(G) straight from PSUM; chunks i,f,o,g
                        a_all = step.tile([128, 4 * pb], fp32, tag=f"aa{d}",
                                          name=f"aa{d}")
                        nc.scalar.activation(
                            a_all[:], ps[:].rearrange("p a b -> p (a b)"),
                            AF.Sigmoid)

                        m2p = step.tile([128, pb], fp32, tag=f"m2p{d}",
                                        name=f"m2p{d}")
                        # m2' = (sig(2g) - 0.5) * sig(i) == i*tanh(g)/2
                        nc.vector.scalar_tensor_tensor(
                            out=m2p[:], in0=a_all[:, 3 * pb:4 * pb],
                            scalar=0.5, in1=a_all[:, 0:pb],
                            op0=ALU.subtract, op1=ALU.mult)
                        if t > 0:
                            m1 = step.tile([128, pb], fp32, tag=f"m1{d}",
                                           name=f"m1{d}")
                            nc.gpsimd.tensor_mul(m1[:],
                                                 a_all[:, pb:2 * pb],
                                                 c_prev[:])
                            nc.vector.scalar_tensor_tensor(
                                out=c_new[:], in0=m2p[:], scalar=2.0,
                                in1=m1[:], op0=ALU.mult, op1=ALU.add)
                        else:
                            nc.vector.tensor_scalar_mul(c_new[:], m2p[:],
                                                        2.0)

                        t_c = step.tile([128, pb], fp32, tag=f"tc{d}",
                                        name=f"tc{d}")
                        nc.scalar.activation(t_c[:], c_new[:], AF.Tanh)
                        if d == "f":
                            nc.vector.tensor_mul(hT["f"][:, tk, :],
                                                 a_all[:, 2 * pb:3 * pb],
                                                 t_c[:])
                        else:
                            nc.gpsimd.tensor_mul(hT["b"][:, tk, :],
                                                 a_all[:, 2 * pb:3 * pb],
                                                 t_c[:])

            # ---- phase 5: emissions into PSUM [(s16,b), blk, T] ----
            from contextlib import ExitStack
            _emctx = ExitStack()
            empool = _emctx.enter_context(
                tc.tile_pool(name="empool", bufs=1, space="PSUM"))
            em_ps = empool.tile([128, nblk, T], fp32, name="emps")
            for k in range(nblk):
                nc.tensor.matmul(
                    em_ps[:, k, :],
                    hT["f"][:].rearrange("p s b -> p (s b)")
                    [:, k * 128:(k + 1) * 128],
                    sb_wof[:],
                    start=True, stop=False, skip_group_check=True,
                )
                nc.tensor.matmul(
                    em_ps[:, k, :],
                    hT["b"][:].rearrange("p s b -> p (s b)")
                    [:, k * 128:(k + 1) * 128],
                    sb_wob[:],
                    start=False, stop=True, skip_group_check=True,
                )

            # stage PSUM -> SBUF, then reshuffle [(s16,b) part, blk, T]
            # -> em_sb [b part, s, T]
            em_stage = big.tile([128, nblk, T], fp32, name="emstage")
            half = (nblk // 2) * T
            nc.vector.tensor_copy(
                em_stage[:].rearrange("p a b -> p (a b)")[:, 0:half],
                em_ps[:].rearrange("p a b -> p (a b)")[:, 0:half])
            nc.scalar.copy(
                em_stage[:].rearrange("p a b -> p (a b)")[:, half:nblk * T],
                em_ps[:].rearrange("p a b -> p (a b)")[:, half:nblk * T])
            _emctx.close()
            pitch = nblk * T
            for s16 in range(s16cnt):
                src_ap = bass.AP(
                    em_stage[:].tensor,
                    em_stage[:].offset + s16 * pb * pitch,
                    [[pitch, pb], [T, nblk], [1, T]],
                )
                dst_ap = bass.AP(
                    em_sb[:].tensor,
                    em_sb[:].offset + s16 * T,
                    [[s_len * T, pb], [s16cnt * T, nblk], [1, T]],
                )
                nc.sync.dma_start(out=dst_ap, in_=src_ap)

            # ---- phase 6: CRF Viterbi forward ----
            nc.vector.tensor_add(score[:, 0, :], em_sb[:, 0, :],
                                 sb_start[:])
            for t in range(1, s_len):
                tmp = crf.tile([pb, T * T], fp32, tag="tmp", name="tmp")
                prev = score[:, t - 1, :].rearrange(
                    "p (o c) -> p o c", o=1).to_broadcast([pb, T, T])
                nc.vector.tensor_tensor(
                    out=tmp[:], in0=prev,
                    in1=sb_trep[:], op=ALU.add)
                mx = crf.tile([pb, T], fp32, tag="mx", name="mx")
                nc.vector.tensor_reduce(
                    out=mx[:],
                    in_=tmp[:].rearrange("p (c q) -> p c q", q=T),
                    axis=AX.X, op=ALU.max)
                nc.vector.tensor_add(score[:, t, :], mx[:],
                                     em_sb[:, t, :])

            nc.sync.dma_start(out=d_scores.ap(), in_=score[:])

    nc.compile()
    return nc


def _prep_host(inputs, dt_np, s_len=S, nblk=NBLK):
    """Build per-core in_maps from full inputs."""
    x = np.asarray(inputs["x"])
    emb = np.asarray(inputs["emb"], dtype=np.float32)
    w_out = np.asarray(inputs["w_out"], dtype=np.float32)
    b_out = np.asarray(inputs["b_out"], dtype=np.float32)
    start = np.asarray(inputs["start"], dtype=np.float32)
    trans = np.asarray(inputs["trans"], dtype=np.float32)

    def perm_rows(w):
        chunks = [w[i * H:(i + 1) * H] for i in range(4)]
        return np.concatenate([chunks[i] for i in GATE_PERM], axis=0)

    shared = {"emb_w": emb.astype(dt_np)}
    for d, (wi, wh, bb) in (("f", ("w_ih_f", "w_hh_f", "b_f")),
                            ("b", ("w_ih_b", "w_hh_b", "b_b"))):
        wih = perm_rows(np.asarray(inputs[wi], dtype=np.float32)).copy()
        whh = perm_rows(np.asarray(inputs[wh], dtype=np.float32)).copy()
        bias = perm_rows(np.asarray(inputs[bb],
                                    dtype=np.float32).reshape(-1, 1))[:, 0]
        bias = bias.copy()
        # tanh-as-sigmoid: scale g-gate (chunk 3) pre-activations by 2
        wih[3 * H:] *= 2.0
        whh[3 * H:] *= 2.0
        bias[3 * H:] *= 2.0
        shared[f"wih_{d}"] = np.ascontiguousarray(wih.T).astype(dt_np)
        shared[f"whh_{d}"] = np.ascontiguousarray(whh.T).astype(dt_np)
        shared[f"biasmat_{d}"] = bias.reshape(4, 128).astype(dt_np)
    ind = np.zeros((4, 4, PB), dtype=np.float32)
    for j in range(4):
        ind[j, j, :] = 1.0
    shared["bias_ind"] = ind.reshape(4, 4 * PB).astype(dt_np)
    shared["wout_f"] = np.ascontiguousarray(w_out[:, :H].T).astype(dt_np)
    shared["wout_b"] = np.ascontiguousarray(w_out[:, H:].T).astype(dt_np)
    shared["ident"] = np.eye(128, dtype=np.float32).astype(dt_np)
    shared["start_t"] = np.tile((start + b_out)[None, :], (PB, 1)).astype(
        np.float32)
    trep = (trans + b_out[None, :]).T.reshape(-1)  # [(c,p)]
    shared["transrep"] = np.tile(trep[None, :], (PB, 1)).astype(np.float32)

    in_maps = []
    for k in range(NCORES):
        xc = x[k * PB:(k + 1) * PB]              # [pb, s]
        ids = np.ascontiguousarray(
            xc.T.reshape(-1).reshape(nblk, 128).T).astype(np.int32)
        idsr = np.ascontiguousarray(
            xc[:, ::-1].T.reshape(-1).reshape(nblk, 128).T).astype(np.int32)
        m = dict(shared)
        m["ids"] = ids
        m["ids_rev"] = idsr
        in_maps.append(m)
    return in_maps


def _host_finalize(scores, trans, end):
    """scores [B, S, T] f32 -> (path int32 [B,S], best f32 [B])."""
    final = scores[:, -1, :] + end[None, :]
    last = np.argmax(final, axis=-1).astype(np.int32)
    best = final.max(axis=-1).astype(np.float32)
    path = np.empty((scores.shape[0], scores.shape[1]), dtype=np.int32)
    path[:, -1] = last
    tag = last
    for t in range(scores.shape[1] - 1, 0, -1):
        val = scores[:, t - 1, :] + trans[:, tag].T   # [B, T(prev)]
        tag = np.argmax(val, axis=-1).astype(np.int32)
        path[:, t - 1] = tag
    return path, best


def _reference_np(inputs):
    """Exact numpy fallback (general mask)."""
    x = np.asarray(inputs["x"])
    mask = np.asarray(inputs["mask"])
    emb = np.asarray(inputs["emb"], np.float32)
    xt = emb[x].transpose(1, 0, 2)

    def lstm(xg, whh):
        h = np.zeros((xg.shape[1], whh.shape[1]), np.float32)
        c = np.zeros_like(h)
        hs = []
        sig = lambda z: 1.0 / (1.0 + np.exp(-z))
        for g_t in xg:
            g = g_t + h @ whh.T
            i, f, gg, o = np.split(g, 4, -1)
            c = sig(f) * c + sig(i) * np.tanh(gg)
            h = sig(o) * np.tanh(c)
            hs.append(h)
        return np.stack(hs)

    xg_f = xt @ np.asarray(inputs["w_ih_f"], np.float32).T + np.asarray(
        inputs["b_f"], np.float32)
    xg_b = xt[::-1] @ np.asarray(inputs["w_ih_b"], np.float32).T + np.asarray(
        inputs["b_b"], np.float32)
    h = np.concatenate([lstm(xg_f, np.asarray(inputs["w_hh_f"], np.float32)),
                        lstm(xg_b, np.asarray(inputs["w_hh_b"],
                                              np.float32))[::-1]], -1)
    em = h @ np.asarray(inputs["w_out"], np.float32).T + np.asarray(
        inputs["b_out"], np.float32)
    trans = np.asarray(inputs["trans"], np.float32)
    m = mask.T
    sc = np.asarray(inputs["start"], np.float32) + em[0]
    hist = []
    for t in range(1, em.shape[0]):
        tot = sc[:, :, None] + trans[None]
        best, idx = tot.max(1), tot.argmax(1).astype(np.int32)
        hist.append(idx)
        sc = np.where(m[t][:, None], best + em[t], sc)
    final = sc + np.asarray(inputs["end"], np.float32)
    last = np.argmax(final, -1).astype(np.int32)
    best = final.max(-1)
    tags = [last]
    tag = last
    bidx = np.arange(x.shape[0])
    for t in range(em.shape[0] - 2, -1, -1):
        prev = hist[t][bidx, tag]
        tag = np.where(m[t + 1], prev, tag)
        tags.append(tag)
    path = np.stack(tags[::-1], 1).astype(np.int32)
    return path, best.astype(np.float32)


def kernel(**inputs):
    mask = np.asarray(inputs["mask"])
    if not mask.all():
        return _reference_np(inputs)

    dt_mm = DT_MM
    key = (S, PB, dt_mm, "f16")
    if key not in _PROGRAM_CACHE:
        _PROGRAM_CACHE[key] = build_program(S, PB, dt_mm, "f16")
    nc = _PROGRAM_CACHE[key]

    from concourse.bass_utils import run_bass_kernel_spmd
    in_maps = _prep_host(inputs, _np_dt(dt_mm))
    res = run_bass_kernel_spmd(nc, in_maps, core_ids=list(range(NCORES)))
    global LAST_RESULT
    LAST_RESULT = res
    scores = np.concatenate([res.results[k]["scores"]
                             for k in range(NCORES)], axis=0)
    trans = np.asarray(inputs["trans"], np.float32)
    end = np.asarray(inputs["end"], np.float32)
    path, best = _host_finalize(scores, trans, end)
    return path, best


# revision 17
# speedup vs baseline: 1.2249x; 1.0413x over previous
"""BiLSTM-CRF Viterbi decode kernel for 8 Trainium2 NeuronCores.

Problem shapes (hardcoded): V=50257, E=128, H=128, T=12, B=64, S=512.

Sharding: data-parallel over batch, 8 sequences per core. Each core runs
the forward and backward LSTM scans with both directions' per-step
elementwise work merged into shared instructions (backward inputs are
stored time-reversed so one slice covers both directions), computes
emissions, and runs the CRF Viterbi forward scan, emitting the per-step
score series. The host does constant prep (bias folding, gate
reordering, tanh-as-sigmoid scaling) and the integer backtrace.
"""

import os

import numpy as np

V, E, H, T, B, S = 50257, 128, 128, 12, 64, 512
NCORES = 8
PB = B // NCORES          # batch per core = 8
NBLK = (S * PB) // 128    # 128-token gather/matmul blocks = 32
G4 = 4 * H                # 512 gate rows per direction
# gate order used on device within one direction: i, f, o, g
# (PyTorch order is i, f, g, o)
GATE_PERM = [0, 1, 3, 2]
# chunk positions in the merged 8-chunk gate tile
POS = {"f": [0, 1, 2, 6], "b": [3, 4, 5, 7]}

_PROGRAM_CACHE = {}
LAST_RESULT = None
DT_MM = os.environ.get("KDT", "f16")


def _np_dt(dt_mm):
    import ml_dtypes
    return {"f32": np.float32, "f16": np.float16,
            "bf16": ml_dtypes.bfloat16}[dt_mm]


def build_program(s_len=S, pb=PB, dt_mm="f16", dt_xg="f16"):
    """Build the Bass/Tile SPMD program for one core."""
    import concourse.bacc as bacc
    import concourse.bass as bass
    import concourse.mybir as mybir
    import concourse.tile as tile

    fp32 = mybir.dt.float32
    DTMM = {"f32": mybir.dt.float32, "f16": mybir.dt.float16,
            "bf16": mybir.dt.bfloat16}[dt_mm]
    DTXG = {"f32": mybir.dt.float32, "f16": mybir.dt.float16,
            "bf16": mybir.dt.bfloat16}[dt_xg]
    AF = mybir.ActivationFunctionType
    ALU = mybir.AluOpType
    AX = mybir.AxisListType

    nblk = (s_len * pb) // 128
    ntok = s_len * pb

    nc = bacc.Bacc("TRN2", target_bir_lowering=False, debug=False)

    # ---- DRAM I/O ----
    d_emb = nc.dram_tensor("emb_w", [V, E], DTMM, kind="ExternalInput")
    d_ids = nc.dram_tensor("ids", [128, nblk], mybir.dt.int32,
                           kind="ExternalInput")
    d_wih = {}
    d_whh = {}
    d_bm = {}
    for d in ("f", "b"):
        d_wih[d] = nc.dram_tensor(f"wih_{d}", [E, G4], DTMM,
                                  kind="ExternalInput")
        d_whh[d] = nc.dram_tensor(f"whh_{d}", [H, G4], DTMM,
                                  kind="ExternalInput")
        d_bm[d] = nc.dram_tensor(f"biasmat_{d}", [4, 128], DTMM,
                                 kind="ExternalInput")
    d_ind = nc.dram_tensor("bias_ind", [4, 4 * pb], DTMM,
                           kind="ExternalInput")
    d_wof = nc.dram_tensor("wout_f", [H, T], DTMM, kind="ExternalInput")
    d_wob = nc.dram_tensor("wout_b", [H, T], DTMM, kind="ExternalInput")
    d_ident = nc.dram_tensor("ident", [128, 128], DTMM, kind="ExternalInput")
    d_start = nc.dram_tensor("start_t", [pb, T], fp32, kind="ExternalInput")
    d_trep = nc.dram_tensor("transrep", [pb, T * T], fp32,
                            kind="ExternalInput")
    d_mx = nc.dram_tensor("mx_out", [pb, s_len, T], fp32,
                          kind="ExternalOutput")
    d_em = nc.dram_tensor("em_out", [pb, s_len, T], fp32,
                          kind="ExternalOutput")

    with tile.TileContext(nc) as tc:
        with (
            tc.tile_pool(name="singles", bufs=1) as singles,
            tc.tile_pool(name="big", bufs=1) as big,
            tc.tile_pool(name="crf", bufs=2) as crf,
        ):
            # ---- load constants ----
            sb_wih = {}
            sb_whh = {}
            sb_bm = {}
            for d in ("f", "b"):
                sb_wih[d] = singles.tile([E, G4], DTMM, name=f"wih{d}")
                nc.sync.dma_start(out=sb_wih[d][:], in_=d_wih[d].ap())
                sb_whh[d] = singles.tile([H, G4], DTMM, name=f"whh{d}")
                nc.sync.dma_start(out=sb_whh[d][:], in_=d_whh[d].ap())
                sb_bm[d] = singles.tile([4, 128], DTMM, name=f"bm{d}")
                nc.sync.dma_start(out=sb_bm[d][:], in_=d_bm[d].ap())
            sb_ind = singles.tile([4, 4 * pb], DTMM, name="ind")
            nc.sync.dma_start(out=sb_ind[:], in_=d_ind.ap())
            sb_wof = singles.tile([H, T], DTMM, name="wof")
            nc.sync.dma_start(out=sb_wof[:], in_=d_wof.ap())
            sb_wob = singles.tile([H, T], DTMM, name="wob")
            nc.sync.dma_start(out=sb_wob[:], in_=d_wob.ap())
            sb_ident = singles.tile([128, 128], DTMM, name="ident")
            nc.sync.dma_start(out=sb_ident[:], in_=d_ident.ap())
            sb_start = singles.tile([pb, T], fp32, name="start")
            nc.sync.dma_start(out=sb_start[:], in_=d_start.ap())
            sb_trep = singles.tile([pb, T * T], fp32, name="trep")
            nc.sync.dma_start(out=sb_trep[:], in_=d_trep.ap())
            sb_ids = singles.tile([128, nblk], mybir.dt.int32, name="ids")
            nc.sync.dma_start(out=sb_ids[:], in_=d_ids.ap())

            # ---- persistent big buffers ----
            hT = {d: big.tile([128, s_len, pb], DTMM, name=f"hT{d}")
                  for d in ("f", "b")}
            em_sb = big.tile([pb, s_len, T], fp32, name="emsb")
            mxs = big.tile([pb, s_len, T], fp32, name="mxs")

            # ---- phase 1+2: gather embeddings, transpose to [E, tok] ----
            with (
                tc.tile_pool(name="gather", bufs=4) as gather,
                tc.tile_pool(name="tps", bufs=2, space="PSUM") as tps,
            ):
                xT = big.tile([128, nblk, 128], DTMM, name="xT")
                for k in range(nblk):
                    ge = gather.tile([128, E], DTMM, tag="ge", name="ge")
                    nc.gpsimd.indirect_dma_start(
                        out=ge[:],
                        out_offset=None,
                        in_=d_emb.ap(),
                        in_offset=bass.IndirectOffsetOnAxis(
                            ap=sb_ids[:, k:k + 1], axis=0),
                    )
                    pt = tps.tile([128, 128], DTMM, tag="pt", name="pt")
                    nc.tensor.transpose(out=pt[:], in_=ge[:],
                                        identity=sb_ident[:])
                    if k % 2 == 0:
                        nc.vector.tensor_copy(xT[:, k, :], pt[:])
                    else:
                        nc.scalar.copy(xT[:, k, :], pt[:])

            # ---- phase 4: fwd+bwd LSTM scans (independent chains) ----
            # Per step and direction, PSUM accumulates
            # bias + W_ih@x_t + W_hh@h_{t-1} for all four gate chunks
            # (order i,f,o,g; g pre-scaled x2 for tanh-as-sigmoid).
            s16cnt = 128 // pb
            with (
                tc.tile_pool(name="gps_f", bufs=2, space="PSUM") as gps_f,
                tc.tile_pool(name="gps_b", bufs=2, space="PSUM") as gps_b,
                tc.tile_pool(name="state", bufs=1) as state,
                tc.tile_pool(name="step", bufs=4) as step,
            ):
                c_tiles = {d: [state.tile([128, pb], fp32, name=f"c{d}{i}")
                               for i in range(2)] for d in ("f", "b")}

                for t in range(s_len):
                    for d in ("f", "b"):
                        tk = t if d == "f" else s_len - 1 - t
                        gp = gps_f if d == "f" else gps_b
                        c_prev = c_tiles[d][(t + 1) % 2]
                        c_new = c_tiles[d][t % 2]

                        ps = gp.tile([128, 4, pb], fp32, tag=f"g{d}",
                                     name=f"g{d}")
                        nc.tensor.matmul(
                            ps[:].rearrange("p a b -> p (a b)"),
                            sb_bm[d][:],
                            sb_ind[:],
                            start=True, stop=False,
                            skip_group_check=True,
                        )
                        xrhs = xT[:, tk // s16cnt,
                                  (tk % s16cnt) * pb:
                                  (tk % s16cnt + 1) * pb]
                        for j in range(4):
                            nc.tensor.matmul(
                                ps[:, j, :],
                                sb_wih[d][:, j * 128:(j + 1) * 128],
                                xrhs,
                                start=False, stop=(t == 0 and j == 3),
                                skip_group_check=True,
                            )
                        if t > 0:
                            h_prev = (hT["f"][:, t - 1, :] if d == "f"
                                      else hT["b"][:, tk + 1, :])
                            for j in range(4):
                                nc.tensor.matmul(
                                    ps[:, j, :],
                                    sb_whh[d][:, j * 128:(j + 1) * 128],
                                    h_prev,
                                    start=False, stop=(j == 3),
                                    skip_group_check=True,
                                )

                        # A = sigmoid(G) straight from PSUM; chunks i,f,o,g
                        a_all = step.tile([128, 4 * pb], fp32, tag=f"aa{d}",
                                          name=f"aa{d}")
                        nc.scalar.activation(
                            a_all[:], ps[:].rearrange("p a b -> p (a b)"),
                            AF.Sigmoid)

                        m2p = step.tile([128, pb], fp32, tag=f"m2p{d}",
                                        name=f"m2p{d}")
                        # m2' = (sig(2g) - 0.5) * sig(i) == i*tanh(g)/2
                        nc.vector.scalar_tensor_tensor(
                            out=m2p[:], in0=a_all[:, 3 * pb:4 * pb],
                            scalar=0.5, in1=a_all[:, 0:pb],
                            op0=ALU.subtract, op1=ALU.mult)
                        if t > 0:
                            m1 = step.tile([128, pb], fp32, tag=f"m1{d}",
                                           name=f"m1{d}")
                            nc.gpsimd.tensor_mul(m1[:],
                                                 a_all[:, pb:2 * pb],
                                                 c_prev[:])
                            nc.vector.scalar_tensor_tensor(
                                out=c_new[:], in0=m2p[:], scalar=2.0,
                                in1=m1[:], op0=ALU.mult, op1=ALU.add)
                        else:
                            nc.vector.tensor_scalar_mul(c_new[:], m2p[:],
                                                        2.0)

                        t_c = step.tile([128, pb], fp32, tag=f"tc{d}",
                                        name=f"tc{d}")
                        nc.scalar.activation(t_c[:], c_new[:], AF.Tanh)
                        if d == "f":
                            nc.vector.tensor_mul(hT["f"][:, tk, :],
                                                 a_all[:, 2 * pb:3 * pb],
                                                 t_c[:])
                        else:
                            nc.gpsimd.tensor_mul(hT["b"][:, tk, :],
                                                 a_all[:, 2 * pb:3 * pb],
                                                 t_c[:])

            # ---- phase 5: emissions into PSUM [(s16,b), blk, T] ----
            from contextlib import ExitStack
            _emctx = ExitStack()
            empool = _emctx.enter_context(
                tc.tile_pool(name="empool", bufs=1, space="PSUM"))
            em_ps = empool.tile([128, nblk, T], fp32, name="emps")
            for k in range(nblk):
                nc.tensor.matmul(
                    em_ps[:, k, :],
                    hT["f"][:].rearrange("p s b -> p (s b)")
                    [:, k * 128:(k + 1) * 128],
                    sb_wof[:],
                    start=True, stop=False, skip_group_check=True,
                )
                nc.tensor.matmul(
                    em_ps[:, k, :],
                    hT["b"][:].rearrange("p s b -> p (s b)")
                    [:, k * 128:(k + 1) * 128],
                    sb_wob[:],
                    start=False, stop=True, skip_group_check=True,
                )

            # stage PSUM -> SBUF, then reshuffle [(s16,b) part, blk, T]
            # -> em_sb [b part, s, T]
            em_stage = big.tile([128, nblk, T], fp32, name="emstage")
            half = (nblk // 2) * T
            nc.vector.tensor_copy(
                em_stage[:].rearrange("p a b -> p (a b)")[:, 0:half],
                em_ps[:].rearrange("p a b -> p (a b)")[:, 0:half])
            nc.scalar.copy(
                em_stage[:].rearrange("p a b -> p (a b)")[:, half:nblk * T],
                em_ps[:].rearrange("p a b -> p (a b)")[:, half:nblk * T])
            _emctx.close()
            pitch = nblk * T
            for s16 in range(s16cnt):
                src_ap = bass.AP(
                    em_stage[:].tensor,
                    em_stage[:].offset + s16 * pb * pitch,
                    [[pitch, pb], [T, nblk], [1, T]],
                )
                dst_ap = bass.AP(
                    em_sb[:].tensor,
                    em_sb[:].offset + s16 * T,
                    [[s_len * T, pb], [s16cnt * T, nblk], [1, T]],
                )
                nc.sync.dma_start(out=dst_ap, in_=src_ap)

            # ---- phase 6: CRF Viterbi forward ----
            # mx[t][b,c] = max_p(mx[t-1][b,p] + em[t-1][b,p] + trans'[c,p])
            # with mx[0] := start'; host reconstructs
            # score[t] = mx[t] + em[t].  emtr = em(bcast c) + trans' is
            # computed off the critical chain on GPSIMD.
            nc.vector.tensor_copy(mxs[:, 0, :], sb_start[:])
            for t in range(1, s_len):
                emtr = crf.tile([pb, T, T], fp32, tag="emtr", name="emtr",
                                bufs=6)
                nc.gpsimd.tensor_add(
                    emtr[:],
                    em_sb[:, t - 1, :].rearrange(
                        "p (o c) -> p o c", o=1).to_broadcast([pb, T, T]),
                    sb_trep[:].rearrange("p (c q) -> p c q", q=T))
                tmp = crf.tile([pb, T * T], fp32, tag="tmp", name="tmp")
                prev = mxs[:, t - 1, :].rearrange(
                    "p (o c) -> p o c", o=1).to_broadcast([pb, T, T])
                nc.vector.tensor_tensor(
                    out=tmp[:].rearrange("p (c q) -> p c q", q=T),
                    in0=prev, in1=emtr[:], op=ALU.add)
                nc.vector.tensor_reduce(
                    out=mxs[:, t, :],
                    in_=tmp[:].rearrange("p (c q) -> p c q", q=T),
                    axis=AX.X, op=ALU.max)

            nc.sync.dma_start(out=d_mx.ap(), in_=mxs[:])
            nc.sync.dma_start(out=d_em.ap(), in_=em_sb[:])

    nc.compile()
    return nc


def _prep_host(inputs, dt_np, s_len=S, nblk=NBLK):
    """Build per-core in_maps from full inputs."""
    x = np.asarray(inputs["x"])
    emb = np.asarray(inputs["emb"], dtype=np.float32)
    w_out = np.asarray(inputs["w_out"], dtype=np.float32)
    b_out = np.asarray(inputs["b_out"], dtype=np.float32)
    start = np.asarray(inputs["start"], dtype=np.float32)
    trans = np.asarray(inputs["trans"], dtype=np.float32)

    def perm_rows(w):
        chunks = [w[i * H:(i + 1) * H] for i in range(4)]
        return np.concatenate([chunks[i] for i in GATE_PERM], axis=0)

    shared = {"emb_w": emb.astype(dt_np)}
    for d, (wi, wh, bb) in (("f", ("w_ih_f", "w_hh_f", "b_f")),
                            ("b", ("w_ih_b", "w_hh_b", "b_b"))):
        wih = perm_rows(np.asarray(inputs[wi], dtype=np.float32)).copy()
        whh = perm_rows(np.asarray(inputs[wh], dtype=np.float32)).copy()
        bias = perm_rows(np.asarray(inputs[bb],
                                    dtype=np.float32).reshape(-1, 1))[:, 0]
        bias = bias.copy()
        # tanh-as-sigmoid: scale g-gate (chunk 3) pre-activations by 2
        wih[3 * H:] *= 2.0
        whh[3 * H:] *= 2.0
        bias[3 * H:] *= 2.0
        shared[f"wih_{d}"] = np.ascontiguousarray(wih.T).astype(dt_np)
        shared[f"whh_{d}"] = np.ascontiguousarray(whh.T).astype(dt_np)
        shared[f"biasmat_{d}"] = bias.reshape(4, 128).astype(dt_np)
    ind = np.zeros((4, 4, PB), dtype=np.float32)
    for j in range(4):
        ind[j, j, :] = 1.0
    shared["bias_ind"] = ind.reshape(4, 4 * PB).astype(dt_np)
    shared["wout_f"] = np.ascontiguousarray(w_out[:, :H].T).astype(dt_np)
    shared["wout_b"] = np.ascontiguousarray(w_out[:, H:].T).astype(dt_np)
    shared["ident"] = np.eye(128, dtype=np.float32).astype(dt_np)
    shared["start_t"] = np.tile((start + b_out)[None, :], (PB, 1)).astype(
        np.float32)
    trep = (trans + b_out[None, :]).T.reshape(-1)  # [(c,p)]
    shared["transrep"] = np.tile(trep[None, :], (PB, 1)).astype(np.float32)

    in_maps = []
    for k in range(NCORES):
        xc = x[k * PB:(k + 1) * PB]              # [pb, s]
        ids = np.ascontiguousarray(
            xc.T.reshape(-1).reshape(nblk, 128).T).astype(np.int32)
        m = dict(shared)
        m["ids"] = ids
        in_maps.append(m)
    return in_maps


def _host_finalize(scores, trans, end):
    """scores [B, S, T] f32 -> (path int32 [B,S], best f32 [B])."""
    final = scores[:, -1, :] + end[None, :]
    last = np.argmax(final, axis=-1).astype(np.int32)
    best = final.max(axis=-1).astype(np.float32)
    path = np.empty((scores.shape[0], scores.shape[1]), dtype=np.int32)
    path[:, -1] = last
    tag = last
    for t in range(scores.shape[1] - 1, 0, -1):
        val = scores[:, t - 1, :] + trans[:, tag].T   # [B, T(prev)]
        tag = np.argmax(val, axis=-1).astype(np.int32)
        path[:, t - 1] = tag
    return path, best


def _reference_np(inputs):
    """Exact numpy fallback (general mask)."""
    x = np.asarray(inputs["x"])
    mask = np.asarray(inputs["mask"])
    emb = np.asarray(inputs["emb"], np.float32)
    xt = emb[x].transpose(1, 0, 2)

    def lstm(xg, whh):
        h = np.zeros((xg.shape[1], whh.shape[1]), np.float32)
        c = np.zeros_like(h)
        hs = []
        sig = lambda z: 1.0 / (1.0 + np.exp(-z))
        for g_t in xg:
            g = g_t + h @ whh.T
            i, f, gg, o = np.split(g, 4, -1)
            c = sig(f) * c + sig(i) * np.tanh(gg)
            h = sig(o) * np.tanh(c)
            hs.append(h)
        return np.stack(hs)

    xg_f = xt @ np.asarray(inputs["w_ih_f"], np.float32).T + np.asarray(
        inputs["b_f"], np.float32)
    xg_b = xt[::-1] @ np.asarray(inputs["w_ih_b"], np.float32).T + np.asarray(
        inputs["b_b"], np.float32)
    h = np.concatenate([lstm(xg_f, np.asarray(inputs["w_hh_f"], np.float32)),
                        lstm(xg_b, np.asarray(inputs["w_hh_b"],
                                              np.float32))[::-1]], -1)
    em = h @ np.asarray(inputs["w_out"], np.float32).T + np.asarray(
        inputs["b_out"], np.float32)
    trans = np.asarray(inputs["trans"], np.float32)
    m = mask.T
    sc = np.asarray(inputs["start"], np.float32) + em[0]
    hist = []
    for t in range(1, em.shape[0]):
        tot = sc[:, :, None] + trans[None]
        best, idx = tot.max(1), tot.argmax(1).astype(np.int32)
        hist.append(idx)
        sc = np.where(m[t][:, None], best + em[t], sc)
    final = sc + np.asarray(inputs["end"], np.float32)
    last = np.argmax(final, -1).astype(np.int32)
    best = final.max(-1)
    tags = [last]
    tag = last
    bidx = np.arange(x.shape[0])
    for t in range(em.shape[0] - 2, -1, -1):
        prev = hist[t][bidx, tag]
        tag = np.where(m[t + 1], prev, tag)
        tags.append(tag)
    path = np.stack(tags[::-1], 1).astype(np.int32)
    return path, best.astype(np.float32)


def kernel(**inputs):
    mask = np.asarray(inputs["mask"])
    if not mask.all():
        return _reference_np(inputs)

    dt_mm = DT_MM
    key = (S, PB, dt_mm, "f16")
    if key not in _PROGRAM_CACHE:
        _PROGRAM_CACHE[key] = build_program(S, PB, dt_mm, "f16")
    nc = _PROGRAM_CACHE[key]

    from concourse.bass_utils import run_bass_kernel_spmd
    in_maps = _prep_host(inputs, _np_dt(dt_mm))
    res = run_bass_kernel_spmd(nc, in_maps, core_ids=list(range(NCORES)))
    global LAST_RESULT
    LAST_RESULT = res
    scores = np.concatenate(
        [res.results[k]["mx_out"] + res.results[k]["em_out"]
         for k in range(NCORES)], axis=0)
    trans = np.asarray(inputs["trans"], np.float32)
    end = np.asarray(inputs["end"], np.float32)
    path, best = _host_finalize(scores, trans, end)
    return path, best
